# revision 1
# baseline (speedup 1.0000x reference)
"""Trainium2 Bass kernel for nn_ClustGeoNodeEncoder (segment_reduce).

Strategy (data-parallel over the cluster axis, per the sharding hint):
  - Host packs the voxel table as [N+1, 8] f32 rows: x, y, z, value,
    onehot(sem==1..4); row N is all-zeros and is the target of padded
    cluster slots.  (count of sem==0 is recovered as n - sum(oh1..4).)
  - Clusters are sorted by length and dealt round-robin to the 8 cores so
    every core compiles the same program (SPMD): 32 tiles x 128 clusters
    per core, tile t padded to Lb[t] = max length in its global rank range.
  - On device, each tile is gathered row-by-row with indirect DMA (one
    instruction gathers one 32B table row for each of the 128 clusters in
    the tile).  Padded slots fetch the zero row, so no masking is needed
    in the moment sums.
  - Pass A (per tile): raw sums / second moments / value stats / semantic
    counts via fused multiply-accumulate (scalar_tensor_tensor accum_out)
    and strided tensor_reduce; centered coordinates are retained in SBUF.
  - Batched per-cluster math on [128, 32] tiles: closed-form symmetric 3x3
    eigenvalues (trig method via Arctan/Sin on the scalar engine),
    principal eigenvector via the spectral projector (A - w0)(A - w1),
    B = A / w2, dirwt = 1 - w1/w2, mode via argmax scan.
  - Pass B (per tile): orientation statistic sc = sum(t * |xc_perp|) from
    the retained centered coords; padded slots contribute a closed-form
    correction term.  Sign-flip + dirwt scaling, then 19 output planes are
    DMA'd out and decoded on the host.
"""

import sys

for _p in ("/opt/trn_rl_repo",):
    if _p not in sys.path:
        sys.path.insert(0, _p)

import numpy as np

N = 2_000_000
C = 32768
L = 256
N_CORES = 8
P = 128
NT = C // (P * N_CORES)  # 32 tiles per core
f32 = np.float32

_PI = float(np.pi)


def _host_prep(data, clust_idx, clust_len):
    data = np.asarray(data, dtype=f32)
    clust_idx = np.asarray(clust_idx).astype(np.int32)
    lens = np.asarray(clust_len).astype(np.int64)

    table = np.zeros((N + 1, 8), dtype=f32)
    table[:N, 0:3] = data[:, 0:3]
    table[:N, 3] = data[:, 4]
    sem = data[:, 5].astype(np.int32)
    for k in range(1, 5):
        table[:N, 3 + k] = (sem == k)

    order = np.argsort(lens, kind="stable")  # ascending length
    # global rank r: tile t = r // (P * N_CORES); slot s = r % (P * N_CORES)
    # core = s % N_CORES ; partition = s // N_CORES
    Lb = np.zeros(NT, dtype=np.int64)
    for t in range(NT):
        Lb[t] = lens[order[t * P * N_CORES:(t + 1) * P * N_CORES]].max()
    S = int(Lb.sum())

    # padded index matrix [C, L] with invalid slots -> N (zero row)
    ar = np.arange(L)[None, :]
    idx_pad = np.where(ar < lens[:, None], clust_idx, N).astype(np.int32)

    idx_blobs = np.zeros((N_CORES, P, S), dtype=np.int32)
    nvecs = np.zeros((N_CORES, P, NT), dtype=f32)
    ids = np.zeros((N_CORES, NT, P), dtype=np.int64)
    off = 0
    for t in range(NT):
        base = t * P * N_CORES
        for core in range(N_CORES):
            sel = order[base + core + N_CORES * np.arange(P)]
            ids[core, t] = sel
            nvecs[core, :, t] = lens[sel]
            idx_blobs[core, :, off:off + Lb[t]] = idx_pad[sel, :Lb[t]]
        off += Lb[t]
    return table, idx_blobs, nvecs, Lb, S, ids


def _build_program(Lb, S):
    import concourse.bass as bass
    import concourse.bacc as bacc
    import concourse.mybir as mybir
    from concourse.tile import TileContext

    dt = mybir.dt
    Alu = mybir.AluOpType
    Act = mybir.ActivationFunctionType

    nc = bacc.Bacc("TRN2", target_bir_lowering=False, debug=False,
                   enable_asserts=False)
    table = nc.dram_tensor("table", [N + 1, 8], dt.float32, kind="ExternalInput")
    idx = nc.dram_tensor("idx", [P, S], dt.int32, kind="ExternalInput")
    nvec_d = nc.dram_tensor("nvec", [P, NT], dt.float32, kind="ExternalInput")
    res = nc.dram_tensor("res", [P, 19 * NT], dt.float32, kind="ExternalOutput")

    TINY = 1e-30

    # tail-hiding: run per-cluster math in groups so earlier groups' DVE/ACT
    # work and output DMAs overlap later groups' gathers on the gpsimd queue
    NG = 8 if NT % 8 == 0 else (2 if NT % 2 == 0 else 1)
    GW = NT // NG

    with TileContext(nc) as tc:
        with tc.tile_pool(name="ret", bufs=1) as ret, \
             tc.tile_pool(name="gp", bufs=3) as gp, \
             tc.tile_pool(name="ip", bufs=3) as ip, \
             tc.tile_pool(name="sp", bufs=2) as sp:

            def full_tile(tag, k=1):
                return ret.tile([P, k * NT], dt.float32, tag=tag, name=tag)

            NV = full_tile("NV")
            RN = full_tile("RN")
            # per-group stat tiles (separate tiles so group 0's math never
            # waits on group 1's gathers through whole-tile dep tracking)
            def gw_tile(tag, g, k=1):
                return ret.tile([P, k * GW], dt.float32, tag=f"{tag}_g{g}",
                                name=f"{tag}_g{g}")
            SUMSg = [gw_tile("SUMS", g, 4) for g in range(NG)]
            OHg = [gw_tile("OH", g, 4) for g in range(NG)]
            PRODg = [gw_tile("PROD", g, 7) for g in range(NG)]
            CENg = [gw_tile("CEN", g, 3) for g in range(NG)]
            SCRAWg = [gw_tile("SCRAW", g) for g in range(NG)]
            # shared output planes (written in absolute column slices)
            MEANV = full_tile("MEANV"); STDV = full_tile("STDV")
            MODE = full_tile("MODE")
            B6 = full_tile("B6", 6)
            V3 = full_tile("V3", 3)

            nc.sync.dma_start(out=NV[:], in_=nvec_d[:, :])
            nc.vector.reciprocal(RN[:], NV[:])

            def tt(op, out, a, b):
                nc.vector.tensor_tensor(out=out, in0=a, in1=b, op=op)

            def ts(out, in0, s, op):
                nc.vector.tensor_scalar(out=out, in0=in0, scalar1=s,
                                        scalar2=None, op0=op)

            def stt(out, in0, s, op0, op1, in1, accum=None):
                nc.vector.scalar_tensor_tensor(out=out, in0=in0, scalar=s,
                                               in1=in1, op0=op0, op1=op1,
                                               accum_out=accum)

            def act(out, in_, func, bias=0.0, scale=1.0):
                nc.scalar.activation(out, in_, func, bias=bias, scale=scale)

            xcs = []
            offs = []
            off = 0
            for t in range(NT):
                offs.append(off)
                off += int(Lb[t])

            def gather_and_pass_a(t):
                g, tl = t // GW, t % GW
                lb = int(Lb[t])
                SUMS, OH, PROD, CEN = SUMSg[g], OHg[g], PRODg[g], CENg[g]
                it = ip.tile([P, lb], dt.int32, tag="idx", name=f"it{t}")
                nc.sync.dma_start(out=it[:], in_=idx[:, offs[t]:offs[t] + lb])
                G = gp.tile([P, lb * 8], dt.float32, tag="G", name=f"G{t}")
                for l in range(lb):
                    nc.gpsimd.indirect_dma_start(
                        out=G[:, l * 8:(l + 1) * 8],
                        out_offset=None,
                        in_=table[:, :],
                        in_offset=bass.IndirectOffsetOnAxis(
                            ap=it[:, l:l + 1], axis=0),
                    )
                Gf = G[:].rearrange("p (l f) -> p f l", f=8)
                nc.vector.tensor_reduce(
                    out=SUMS[:].rearrange("p (f t) -> p f t", t=GW)[:, :, tl],
                    in_=Gf[:, 0:4, :], axis=mybir.AxisListType.X, op=Alu.add)
                nc.vector.tensor_reduce(
                    out=OH[:].rearrange("p (f t) -> p f t", t=GW)[:, :, tl],
                    in_=Gf[:, 4:8, :], axis=mybir.AxisListType.X, op=Alu.add)
                scratch = sp.tile([P, lb], dt.float32, tag="scr", name=f"scr{t}")
                pairs = [(0, 0), (0, 1), (0, 2), (1, 1), (1, 2), (2, 2), (3, 3)]
                for q, (i, j) in enumerate(pairs):
                    nc.vector.scalar_tensor_tensor(
                        out=scratch[:],
                        in0=Gf[:, i, :], scalar=1.0, in1=Gf[:, j, :],
                        op0=Alu.mult, op1=Alu.mult,
                        accum_out=PROD[:, q * GW + tl:q * GW + tl + 1])
                nc.vector.tensor_scalar(
                    out=CEN[:].rearrange("p (f t) -> p f t", t=GW)[:, :, tl],
                    in0=SUMS[:].rearrange("p (f t) -> p f t", t=GW)[:, 0:3, tl],
                    scalar1=RN[:, t:t + 1], scalar2=None, op0=Alu.mult)
                xc = ret.tile([P, 3 * lb], dt.float32, tag=f"xc{t}", name=f"xc{t}")
                for i in range(3):
                    nc.vector.tensor_scalar(
                        out=xc[:, i * lb:(i + 1) * lb],
                        in0=Gf[:, i, :],
                        scalar1=CEN[:, i * GW + tl:i * GW + tl + 1],
                        scalar2=None, op0=Alu.subtract)
                xcs.append(xc)

            def cluster_math(g):
                g0, g1 = g * GW, (g + 1) * GW
                SUMS, OH, PROD, CEN = SUMSg[g], OHg[g], PRODg[g], CENg[g]
                SCRAW = SCRAWg[g]
                NVv, RNv = NV[:, g0:g1], RN[:, g0:g1]

                def tmp(tag, k=1):
                    return ret.tile([P, k * GW], dt.float32,
                                    tag=f"{tag}_g{g}", name=f"{tag}_g{g}")

                def sl(T, i):   # slice i of a local k*GW tile
                    return T[:, i * GW:(i + 1) * GW]

                def osl(T, i):  # slice of a shared k*NT output tile
                    return T[:, i * NT + g0:i * NT + g1]

                A = tmp("A", 6)
                cmap = [(0, 0, 0), (1, 0, 1), (2, 0, 2), (3, 1, 1), (4, 1, 2),
                        (5, 2, 2)]
                SC1 = tmp("SC1")
                for q, i, j in cmap:
                    tt(Alu.mult, SC1[:], sl(CEN, i), sl(SUMS, j))
                    tt(Alu.subtract, sl(A, q), sl(PROD, q), SC1[:])

                # value stats
                VAR = tmp("VAR"); NM1 = tmp("NM1")
                tt(Alu.mult, osl(MEANV, 0), sl(SUMS, 3), RNv)
                tt(Alu.mult, VAR[:], osl(MEANV, 0), sl(SUMS, 3))
                tt(Alu.subtract, VAR[:], sl(PROD, 6), VAR[:])
                ts(NM1[:], NVv, 1.0, Alu.subtract)
                nc.vector.reciprocal(SC1[:], NM1[:])
                tt(Alu.mult, VAR[:], VAR[:], SC1[:])
                ts(VAR[:], VAR[:], 0.0, Alu.max)
                act(osl(STDV, 0), VAR[:], Act.Sqrt)

                BEST = tmp("BEST"); GT = tmp("GT"); KT = tmp("KT")

                # eigenvalues: trig closed form
                Q = tmp("Q"); P1 = tmp("P1"); P2 = tmp("P2"); PP = tmp("PP")
                RP = tmp("RP"); DET = tmp("DET"); RR = tmp("RR"); SS = tmp("SS")
                AT = tmp("AT"); PHI = tmp("PHI")
                W0 = tmp("W0"); W1 = tmp("W1"); W2 = tmp("W2"); RW2 = tmp("RW2")
                DIRWT = tmp("DIRWT")
                NB = tmp("NB", 6)

                tt(Alu.add, Q[:], sl(A, 0), sl(A, 3))
                tt(Alu.add, Q[:], Q[:], sl(A, 5))
                ts(Q[:], Q[:], 1.0 / 3.0, Alu.mult)

                tt(Alu.mult, P1[:], sl(A, 1), sl(A, 1))
                tt(Alu.mult, SC1[:], sl(A, 2), sl(A, 2))
                tt(Alu.add, P1[:], P1[:], SC1[:])
                tt(Alu.mult, SC1[:], sl(A, 4), sl(A, 4))
                tt(Alu.add, P1[:], P1[:], SC1[:])

                BD = tmp("BD", 3)
                tt(Alu.subtract, sl(BD, 0), sl(A, 0), Q[:])
                tt(Alu.subtract, sl(BD, 1), sl(A, 3), Q[:])
                tt(Alu.subtract, sl(BD, 2), sl(A, 5), Q[:])
                tt(Alu.mult, P2[:], sl(BD, 0), sl(BD, 0))
                tt(Alu.mult, SC1[:], sl(BD, 1), sl(BD, 1))
                tt(Alu.add, P2[:], P2[:], SC1[:])
                tt(Alu.mult, SC1[:], sl(BD, 2), sl(BD, 2))
                tt(Alu.add, P2[:], P2[:], SC1[:])
                stt(P2[:], P1[:], 2.0, Alu.mult, Alu.add, P2[:])
                ts(PP[:], P2[:], 1.0 / 6.0, Alu.mult)
                act(PP[:], PP[:], Act.Sqrt)
                ts(SC1[:], PP[:], TINY, Alu.max)
                nc.vector.reciprocal(RP[:], SC1[:])

                tt(Alu.mult, sl(NB, 0), sl(BD, 0), RP[:])
                tt(Alu.mult, sl(NB, 1), sl(A, 1), RP[:])
                tt(Alu.mult, sl(NB, 2), sl(A, 2), RP[:])
                tt(Alu.mult, sl(NB, 3), sl(BD, 1), RP[:])
                tt(Alu.mult, sl(NB, 4), sl(A, 4), RP[:])
                tt(Alu.mult, sl(NB, 5), sl(BD, 2), RP[:])

                SC2 = tmp("SC2"); SC3 = tmp("SC3")
                tt(Alu.mult, SC1[:], sl(NB, 3), sl(NB, 5))
                tt(Alu.mult, SC2[:], sl(NB, 4), sl(NB, 4))
                tt(Alu.subtract, SC1[:], SC1[:], SC2[:])
                tt(Alu.mult, DET[:], sl(NB, 0), SC1[:])
                tt(Alu.mult, SC1[:], sl(NB, 1), sl(NB, 5))
                tt(Alu.mult, SC2[:], sl(NB, 4), sl(NB, 2))
                tt(Alu.subtract, SC1[:], SC1[:], SC2[:])
                tt(Alu.mult, SC1[:], sl(NB, 1), SC1[:])
                tt(Alu.subtract, DET[:], DET[:], SC1[:])
                tt(Alu.mult, SC1[:], sl(NB, 1), sl(NB, 4))
                tt(Alu.mult, SC2[:], sl(NB, 3), sl(NB, 2))
                tt(Alu.subtract, SC1[:], SC1[:], SC2[:])
                tt(Alu.mult, SC1[:], sl(NB, 2), SC1[:])
                tt(Alu.add, DET[:], DET[:], SC1[:])

                ts(RR[:], DET[:], 0.5, Alu.mult)
                ts(RR[:], RR[:], -1.0, Alu.max)
                ts(RR[:], RR[:], 1.0, Alu.min)
                tt(Alu.mult, SS[:], RR[:], RR[:])
                nc.vector.tensor_scalar(out=SS[:], in0=SS[:], scalar1=-1.0,
                                        scalar2=1.0, op0=Alu.mult, op1=Alu.add)
                ts(SS[:], SS[:], 0.0, Alu.max)
                act(SS[:], SS[:], Act.Sqrt)
                UA = tmp("UA"); UB = tmp("UB")
                ts(SC1[:], RR[:], -1.0, Alu.mult)
                tt(Alu.max, SC1[:], SC1[:], RR[:])
                ts(SS[:], SS[:], TINY, Alu.max)
                nc.vector.reciprocal(SC2[:], SS[:])
                tt(Alu.mult, UA[:], SC1[:], SC2[:])
                ts(SC1[:], UA[:], TINY, Alu.max)
                nc.vector.reciprocal(UB[:], SC1[:])
                tt(Alu.min, SC2[:], UA[:], UB[:])
                act(SC2[:], SC2[:], Act.Arctan)
                ts(SC1[:], UA[:], 1.0, Alu.is_gt)
                nc.vector.tensor_scalar(out=SC3[:], in0=SC2[:], scalar1=-2.0,
                                        scalar2=_PI / 2.0, op0=Alu.mult,
                                        op1=Alu.add)
                tt(Alu.mult, SC3[:], SC3[:], SC1[:])
                tt(Alu.add, SC2[:], SC2[:], SC3[:])
                ts(SC3[:], RR[:], 0.0, Alu.is_lt)
                nc.vector.tensor_scalar(out=SC3[:], in0=SC3[:], scalar1=-2.0,
                                        scalar2=1.0, op0=Alu.mult, op1=Alu.add)
                tt(Alu.mult, AT[:], SC2[:], SC3[:])
                nc.vector.tensor_scalar(out=PHI[:], in0=AT[:],
                                        scalar1=-1.0 / 3.0,
                                        scalar2=_PI / 6.0 + _PI / 2.0,
                                        op0=Alu.mult, op1=Alu.add)
                act(SC1[:], PHI[:], Act.Sin)
                tt(Alu.mult, SC1[:], SC1[:], PP[:])
                stt(W2[:], SC1[:], 2.0, Alu.mult, Alu.add, Q[:])
                nc.vector.tensor_scalar(out=PHI[:], in0=AT[:],
                                        scalar1=-1.0 / 3.0,
                                        scalar2=_PI / 6.0 + _PI / 6.0,
                                        op0=Alu.mult, op1=Alu.add)
                act(SC1[:], PHI[:], Act.Sin)
                tt(Alu.mult, SC1[:], SC1[:], PP[:])
                stt(W0[:], SC1[:], -2.0, Alu.mult, Alu.add, Q[:])
                ts(SC1[:], Q[:], 3.0, Alu.mult)
                tt(Alu.subtract, W1[:], SC1[:], W0[:])
                tt(Alu.subtract, W1[:], W1[:], W2[:])

                ts(SC1[:], W2[:], TINY, Alu.max)
                nc.vector.reciprocal(RW2[:], SC1[:])
                tt(Alu.mult, DIRWT[:], W1[:], RW2[:])
                nc.vector.tensor_scalar(out=DIRWT[:], in0=DIRWT[:],
                                        scalar1=-1.0, scalar2=1.0,
                                        op0=Alu.mult, op1=Alu.add)
                for q in range(6):
                    tt(Alu.mult, osl(B6, q), sl(A, q), RW2[:])

                CD = tmp("CD", 3)
                DD = tmp("DD", 3)
                for qi, ai in enumerate((0, 3, 5)):
                    tt(Alu.subtract, sl(CD, qi), sl(A, ai), W0[:])
                    tt(Alu.subtract, sl(DD, qi), sl(A, ai), W1[:])
                M9 = tmp("M9", 9)

                def mcol(colq, dv):
                    crow = [(sl(CD, 0), sl(A, 1), sl(A, 2)),
                            (sl(A, 1), sl(CD, 1), sl(A, 4)),
                            (sl(A, 2), sl(A, 4), sl(CD, 2))]
                    for r in range(3):
                        a0, a1, a2 = crow[r]
                        tt(Alu.mult, SC1[:], a0, dv[0])
                        tt(Alu.mult, SC2[:], a1, dv[1])
                        tt(Alu.add, SC1[:], SC1[:], SC2[:])
                        tt(Alu.mult, SC2[:], a2, dv[2])
                        tt(Alu.add, sl(M9, colq * 3 + r), SC1[:], SC2[:])

                mcol(0, (sl(DD, 0), sl(A, 1), sl(A, 2)))
                mcol(1, (sl(A, 1), sl(DD, 1), sl(A, 4)))
                mcol(2, (sl(A, 2), sl(A, 4), sl(DD, 2)))

                CN = tmp("CN", 3)
                for j in range(3):
                    tt(Alu.mult, sl(CN, j), sl(M9, j * 3), sl(M9, j * 3))
                    tt(Alu.mult, SC1[:], sl(M9, j * 3 + 1), sl(M9, j * 3 + 1))
                    tt(Alu.add, sl(CN, j), sl(CN, j), SC1[:])
                    tt(Alu.mult, SC1[:], sl(M9, j * 3 + 2), sl(M9, j * 3 + 2))
                    tt(Alu.add, sl(CN, j), sl(CN, j), SC1[:])
                NBEST = tmp("NBEST")
                for i in range(3):
                    nc.vector.tensor_copy(out=osl(V3, i), in_=sl(M9, i))
                nc.vector.tensor_copy(out=NBEST[:], in_=sl(CN, 0))
                for j in (1, 2):
                    tt(Alu.is_gt, GT[:], sl(CN, j), NBEST[:])
                    for i in range(3):
                        tt(Alu.subtract, SC1[:], sl(M9, j * 3 + i), osl(V3, i))
                        tt(Alu.mult, SC1[:], SC1[:], GT[:])
                        tt(Alu.add, osl(V3, i), osl(V3, i), SC1[:])
                    tt(Alu.max, NBEST[:], NBEST[:], sl(CN, j))
                ts(SC1[:], NBEST[:], 1e-37, Alu.max)
                act(SC2[:], SC1[:], Act.Sqrt)
                nc.vector.reciprocal(SC2[:], SC2[:])
                for i in range(3):
                    tt(Alu.mult, osl(V3, i), osl(V3, i), SC2[:])

                # mode of semantic class (ties -> smallest)
                tt(Alu.subtract, BEST[:], NVv, sl(OH, 0))
                for k in (1, 2, 3):
                    tt(Alu.subtract, BEST[:], BEST[:], sl(OH, k))
                nc.vector.memset(osl(MODE, 0), 0.0)
                for k in range(1, 5):
                    ck = sl(OH, k - 1)
                    tt(Alu.is_gt, GT[:], ck, BEST[:])
                    nc.vector.tensor_scalar(out=KT[:], in0=osl(MODE, 0),
                                            scalar1=-1.0, scalar2=float(k),
                                            op0=Alu.mult, op1=Alu.add)
                    tt(Alu.mult, KT[:], KT[:], GT[:])
                    tt(Alu.add, osl(MODE, 0), osl(MODE, 0), KT[:])
                    tt(Alu.max, BEST[:], BEST[:], ck)
                return DIRWT

            def pass_b(t):
                lb = int(Lb[t])
                xc = xcs[t]
                g = t // GW
                xcx = xc[:, 0:lb]; xcy = xc[:, lb:2 * lb]
                xcz = xc[:, 2 * lb:3 * lb]
                T = sp.tile([P, lb], dt.float32, tag="T", name=f"T{t}")
                S2 = sp.tile([P, lb], dt.float32, tag="S2", name=f"S2_{t}")
                S2b = sp.tile([P, lb], dt.float32, tag="S2b", name=f"S2b{t}")
                R = sp.tile([P, lb], dt.float32, tag="R", name=f"R{t}")
                nc.vector.tensor_scalar(out=T[:], in0=xcx,
                                        scalar1=V3[:, 0 * NT + t:0 * NT + t + 1],
                                        scalar2=None, op0=Alu.mult)
                stt(T[:], xcy, V3[:, 1 * NT + t:1 * NT + t + 1],
                    Alu.mult, Alu.add, T[:])
                stt(T[:], xcz, V3[:, 2 * NT + t:2 * NT + t + 1],
                    Alu.mult, Alu.add, T[:])
                stt(S2[:], xcx, 1.0, Alu.mult, Alu.mult, xcx)
                stt(S2b[:], xcy, 1.0, Alu.mult, Alu.mult, xcy)
                tt(Alu.add, S2[:], S2[:], S2b[:])
                stt(S2b[:], xcz, 1.0, Alu.mult, Alu.mult, xcz)
                tt(Alu.add, S2[:], S2[:], S2b[:])
                stt(S2b[:], T[:], 1.0, Alu.mult, Alu.mult, T[:])
                tt(Alu.subtract, S2[:], S2[:], S2b[:])
                ts(S2[:], S2[:], 0.0, Alu.max)
                act(R[:], S2[:], Act.Sqrt)
                stt(S2b[:], T[:], 1.0, Alu.mult, Alu.mult, R[:],
                    accum=SCRAWg[g][:, t % GW:t % GW + 1])

            def sign_phase(g, DIRWT):
                g0, g1 = g * GW, (g + 1) * GW
                CEN = CENg[g]; SCRAW = SCRAWg[g]
                NVv = NV[:, g0:g1]

                def tmp(tag, k=1):
                    return ret.tile([P, k * GW], dt.float32,
                                    tag=f"{tag}_g{g}", name=f"{tag}_g{g}")

                def sl(T, i):
                    return T[:, i * GW:(i + 1) * GW]

                def osl(T, i):
                    return T[:, i * NT + g0:i * NT + g1]

                T0 = tmp("T0"); CC = tmp("CC"); R0 = tmp("R0")
                SCV = tmp("SCV"); FAC = tmp("FAC"); SC9 = tmp("SC9")
                GT9 = tmp("GT9"); NPAD = tmp("NPAD")
                tt(Alu.mult, T0[:], sl(CEN, 0), osl(V3, 0))
                tt(Alu.mult, SC9[:], sl(CEN, 1), osl(V3, 1))
                tt(Alu.add, T0[:], T0[:], SC9[:])
                tt(Alu.mult, SC9[:], sl(CEN, 2), osl(V3, 2))
                tt(Alu.add, T0[:], T0[:], SC9[:])
                ts(T0[:], T0[:], -1.0, Alu.mult)
                tt(Alu.mult, CC[:], sl(CEN, 0), sl(CEN, 0))
                tt(Alu.mult, SC9[:], sl(CEN, 1), sl(CEN, 1))
                tt(Alu.add, CC[:], CC[:], SC9[:])
                tt(Alu.mult, SC9[:], sl(CEN, 2), sl(CEN, 2))
                tt(Alu.add, CC[:], CC[:], SC9[:])
                tt(Alu.mult, SC9[:], T0[:], T0[:])
                tt(Alu.subtract, R0[:], CC[:], SC9[:])
                ts(R0[:], R0[:], 0.0, Alu.max)
                act(R0[:], R0[:], Act.Sqrt)
                for t in range(g0, g1):
                    nc.vector.tensor_scalar(
                        out=NPAD[:, t - g0:t - g0 + 1],
                        in0=NV[:, t:t + 1], scalar1=-1.0,
                        scalar2=float(int(Lb[t])), op0=Alu.mult, op1=Alu.add)
                tt(Alu.mult, SC9[:], T0[:], R0[:])
                tt(Alu.mult, SC9[:], SC9[:], NPAD[:])
                tt(Alu.subtract, SCV[:], SCRAW[:], SC9[:])
                ts(GT9[:], SCV[:], 0.0, Alu.is_lt)
                nc.vector.tensor_scalar(out=GT9[:], in0=GT9[:], scalar1=-2.0,
                                        scalar2=1.0, op0=Alu.mult, op1=Alu.add)
                tt(Alu.mult, FAC[:], DIRWT[:], GT9[:])
                for i in range(3):
                    tt(Alu.mult, osl(V3, i), osl(V3, i), FAC[:])
                # group-local output DMAs for fully-final planes
                for j, pl in [(0, sl(CEN, 0)), (1, sl(CEN, 1)), (2, sl(CEN, 2)),
                              (3, osl(B6, 0)), (4, osl(B6, 1)), (5, osl(B6, 2)),
                              (6, osl(B6, 1)), (7, osl(B6, 3)), (8, osl(B6, 4)),
                              (9, osl(B6, 2)), (10, osl(B6, 4)), (11, osl(B6, 5)),
                              (12, osl(V3, 0)), (13, osl(V3, 1)), (14, osl(V3, 2)),
                              (15, NVv), (16, osl(MEANV, 0)), (17, osl(STDV, 0)),
                              (18, osl(MODE, 0))]:
                    nc.sync.dma_start(out=res[:, j * NT + g0:j * NT + g1],
                                      in_=pl)

            # schedule: gathers for all tiles in order; after each group's
            # last tile, its cluster math + pass B + sign phase (overlaps the
            # next group's gathers on the gpsimd queue)
            for g in range(NG):
                for t in range(g * GW, (g + 1) * GW):
                    gather_and_pass_a(t)
                DIRWT = cluster_math(g)
                for t in range(g * GW, (g + 1) * GW):
                    pass_b(t)
                sign_phase(g, DIRWT)

    nc.compile()
    return nc


_cache = {}
_last = None


def kernel(data, clust_idx, clust_len):
    global N, C, L, NT
    data = np.asarray(data)
    clust_idx = np.asarray(clust_idx)
    N = int(data.shape[0])
    C, L = int(clust_idx.shape[0]), int(clust_idx.shape[1])
    assert C % (P * N_CORES) == 0, f"cluster count {C} not divisible by {P * N_CORES}"
    NT = C // (P * N_CORES)
    table, idx_blobs, nvecs, Lb, S, ids = _host_prep(data, clust_idx, clust_len)

    key = tuple(int(x) for x in Lb)
    if key not in _cache:
        _cache[key] = _build_program(Lb, S)
    nc = _cache[key]

    from concourse.bass_utils import run_bass_kernel_spmd
    in_maps = [{"table": table, "idx": idx_blobs[c], "nvec": nvecs[c]}
               for c in range(N_CORES)]
    global _last
    _last = (nc, in_maps)
    res = run_bass_kernel_spmd(nc, in_maps, list(range(N_CORES)))

    out = np.zeros((C, 19), dtype=f32)
    for core in range(N_CORES):
        r = res.results[core]["res"].reshape(P, 19, NT)
        for t in range(NT):
            out[ids[core, t]] = r[:, :, t]
    return out



# revision 4
# speedup vs baseline: 1.3800x; 1.3800x over previous
"""Trainium2 Bass kernel for nn_ClustGeoNodeEncoder (segment_reduce).

Strategy (data-parallel over the cluster axis, per the sharding hint):
  - Host packs the voxel table as [N+4, 8] f32 rows: x, y, z, value,
    onehot(sem==1..4); the table rows are PERMUTED so that each cluster's
    members tend to sit at consecutive positions (greedy claim pass), and
    the last 4 rows are zeros (targets of padded offsets).
  - Each cluster's member list is grouped into chains of up to 4
    consecutive permuted positions.  One indirect-DMA offset fetches a
    whole chain (payload 4 rows = 128B), cutting the dominant cost — the
    ~1.4us INDIRECT1D instruction per 128 offsets — by ~30%.
  - Chain slots beyond the chain length are zeroed on-device with
    (chainlen > k) masks, making them exact zero rows, so the moment sums
    need no other masking and the padded-slot closed-form correction in
    the orientation pass applies unchanged.
  - Clusters are sorted by offset count and dealt round-robin to the 8
    cores (SPMD): 32 tiles x 128 clusters per core, tile t padded to
    Ob[t] = max offsets in its rank range.
  - Pass A (per tile): raw sums / second moments / value stats / semantic
    counts via fused multiply-accumulate and strided tensor_reduce;
    centered coordinates are retained in SBUF (per-group recycled pool).
  - Batched per-cluster math on [128, GW] tiles: closed-form symmetric
    3x3 eigenvalues (trig method), principal eigenvector via the spectral
    projector, B = A / w2, dirwt = 1 - w1/w2, mode via argmax scan.
  - Pass B (per tile): orientation statistic from the retained centered
    coords; zero slots contribute a closed-form correction term.
"""

import sys

for _p in ("/opt/trn_rl_repo",):
    if _p not in sys.path:
        sys.path.insert(0, _p)

import numpy as np

N = 2_000_000
C = 32768
L = 256
N_CORES = 8
P = 128
NT = C // (P * N_CORES)  # 32 tiles per core
F = 4                    # chain length (rows per indirect offset)
f32 = np.float32

_PI = float(np.pi)


def _host_prep(data, clust_idx, clust_len):
    data = np.asarray(data, dtype=f32)
    clust_idx = np.asarray(clust_idx).astype(np.int64)
    lens = np.asarray(clust_len).astype(np.int64)

    base = np.zeros((N, 8), dtype=f32)
    base[:, 0:3] = data[:, 0:3]
    base[:, 3] = data[:, 4]
    sem = data[:, 5].astype(np.int32)
    for k in range(1, 5):
        base[:, 3 + k] = (sem == k)

    # --- claim pass: greedy per-cluster runs -> table permutation pos[] ---
    rng = np.random.default_rng(0)
    order_c = rng.permutation(C)
    claimed = np.zeros(N, bool)
    pos = np.full(N, -1, np.int64)
    nxt = 0
    for c in order_c:
        rows = np.unique(clust_idx[c, :lens[c]])
        un = rows[~claimed[rows]]
        claimed[un] = True
        pos[un] = nxt + np.arange(len(un))
        nxt += len(un)
    rest = np.where(~claimed)[0]
    pos[rest] = nxt + np.arange(len(rest))

    table = np.zeros((N + F, 8), dtype=f32)
    table[pos] = base

    # --- chain grouping per cluster ---
    starts_c = []
    clens_c = []
    o_c = np.zeros(C, np.int64)
    for c in range(C):
        q = np.sort(pos[clust_idx[c, :lens[c]]])
        n = len(q)
        st = []
        cl = []
        i = 0
        while i < n:
            j = i + 1
            while j < n and j - i < F and q[j] == q[i] + (j - i):
                j += 1
            st.append(q[i])
            cl.append(j - i)
            i = j
        starts_c.append(np.asarray(st, np.int32))
        clens_c.append(np.asarray(cl, f32))
        o_c[c] = len(st)

    # --- tile packing: sort by offset count, deal round-robin ---
    order = np.argsort(o_c, kind="stable")
    Ob = np.zeros(NT, dtype=np.int64)
    for t in range(NT):
        Ob[t] = o_c[order[t * P * N_CORES:(t + 1) * P * N_CORES]].max()
    S = int(Ob.sum())

    idx_blobs = np.full((N_CORES, P, S), N, dtype=np.int32)
    clen_blobs = np.full((N_CORES, P, S), float(F), dtype=f32)
    nvecs = np.zeros((N_CORES, P, NT), dtype=f32)
    ids = np.zeros((N_CORES, NT, P), dtype=np.int64)
    off = 0
    for t in range(NT):
        tb = t * P * N_CORES
        for core in range(N_CORES):
            sel = order[tb + core + N_CORES * np.arange(P)]
            ids[core, t] = sel
            nvecs[core, :, t] = lens[sel]
            for p in range(P):
                c = sel[p]
                o = len(starts_c[c])
                idx_blobs[core, p, off:off + o] = starts_c[c]
                clen_blobs[core, p, off:off + o] = clens_c[c]
        off += Ob[t]
    return table, idx_blobs, clen_blobs, nvecs, Ob, S, ids


def _build_program(Ob, S):
    import concourse.bass as bass
    import concourse.bacc as bacc
    import concourse.mybir as mybir
    from concourse.tile import TileContext

    dt = mybir.dt
    Alu = mybir.AluOpType
    Act = mybir.ActivationFunctionType

    nc = bacc.Bacc("TRN2", target_bir_lowering=False, debug=False,
                   enable_asserts=False)
    table = nc.dram_tensor("table", [N + F, 8], dt.float32,
                           kind="ExternalInput")
    idx = nc.dram_tensor("idx", [P, S], dt.int32, kind="ExternalInput")
    clen_d = nc.dram_tensor("clen", [P, S], dt.float32, kind="ExternalInput")
    nvec_d = nc.dram_tensor("nvec", [P, NT], dt.float32, kind="ExternalInput")
    res = nc.dram_tensor("res", [P, 19 * NT], dt.float32,
                         kind="ExternalOutput")

    TINY = 1e-30

    NG = 8 if NT % 8 == 0 else (2 if NT % 2 == 0 else 1)
    GW = NT // NG

    with TileContext(nc) as tc:
        with tc.tile_pool(name="ret", bufs=1) as ret, \
             tc.tile_pool(name="gp", bufs=2) as gp, \
             tc.tile_pool(name="xp", bufs=1) as xp, \
             tc.tile_pool(name="sp", bufs=1) as sp:

            def full_tile(tag, k=1):
                return ret.tile([P, k * NT], dt.float32, tag=tag, name=tag)

            NV = full_tile("NV")
            RN = full_tile("RN")

            def gw_tile(tag, g, k=1):
                return ret.tile([P, k * GW], dt.float32, tag=f"{tag}_g{g}",
                                name=f"{tag}_g{g}")
            SUMSg = [gw_tile("SUMS", g, 4) for g in range(NG)]
            OHg = [gw_tile("OH", g, 4) for g in range(NG)]
            PRODg = [gw_tile("PROD", g, 7) for g in range(NG)]
            CENg = [gw_tile("CEN", g, 3) for g in range(NG)]
            SCRAWg = [gw_tile("SCRAW", g) for g in range(NG)]
            MEANV = full_tile("MEANV"); STDV = full_tile("STDV")
            MODE = full_tile("MODE")
            B6 = full_tile("B6", 6)
            V3 = full_tile("V3", 3)

            # whole-run preloads: offsets + chain lengths + cluster sizes
            ITALL = ret.tile([P, S], dt.int32, tag="ITALL", name="ITALL")
            CLALL = ret.tile([P, S], dt.float32, tag="CLALL", name="CLALL")
            nc.sync.dma_start(out=ITALL[:], in_=idx[:, :])
            nc.sync.dma_start(out=CLALL[:], in_=clen_d[:, :])
            nc.sync.dma_start(out=NV[:], in_=nvec_d[:, :])
            nc.vector.reciprocal(RN[:], NV[:])

            def tt(op, out, a, b):
                nc.vector.tensor_tensor(out=out, in0=a, in1=b, op=op)

            def ts(out, in0, s, op):
                nc.vector.tensor_scalar(out=out, in0=in0, scalar1=s,
                                        scalar2=None, op0=op)

            def stt(out, in0, s, op0, op1, in1, accum=None):
                nc.vector.scalar_tensor_tensor(out=out, in0=in0, scalar=s,
                                               in1=in1, op0=op0, op1=op1,
                                               accum_out=accum)

            def act(out, in_, func, bias=0.0, scale=1.0):
                nc.scalar.activation(out, in_, func, bias=bias, scale=scale)

            xcs = []
            offs = []
            off = 0
            for t in range(NT):
                offs.append(off)
                off += int(Ob[t])

            def gather_and_pass_a(t):
                g, tl = t // GW, t % GW
                ob = int(Ob[t])
                ls = F * ob          # slot count for this tile
                SUMS, OH, PROD, CEN = SUMSg[g], OHg[g], PRODg[g], CENg[g]
                G = gp.tile([P, ls * 8], dt.float32, tag="G", name=f"G{t}")
                for l in range(ob):
                    nc.gpsimd.indirect_dma_start(
                        out=G[:, l * F * 8:(l + 1) * F * 8],
                        out_offset=None,
                        in_=table[:, :],
                        in_offset=bass.IndirectOffsetOnAxis(
                            ap=ITALL[:, offs[t] + l:offs[t] + l + 1], axis=0),
                    )
                # zero chain slots beyond chainlen: slot k valid iff cl > k
                Gs = G[:].rearrange("p (o w) -> p o w", w=F * 8)
                MK = sp.tile([P, ob], dt.float32, tag="MK", name=f"MK{t}")
                for k in range(1, F):
                    ts(MK[:], CLALL[:, offs[t]:offs[t] + ob], float(k),
                       Alu.is_gt)
                    for f in range(8):
                        tt(Alu.mult, Gs[:, :, k * 8 + f], Gs[:, :, k * 8 + f],
                           MK[:])
                Gf = G[:].rearrange("p (l f) -> p f l", f=8)
                nc.vector.tensor_reduce(
                    out=SUMS[:].rearrange("p (f t) -> p f t", t=GW)[:, :, tl],
                    in_=Gf[:, 0:4, :], axis=mybir.AxisListType.X, op=Alu.add)
                nc.vector.tensor_reduce(
                    out=OH[:].rearrange("p (f t) -> p f t", t=GW)[:, :, tl],
                    in_=Gf[:, 4:8, :], axis=mybir.AxisListType.X, op=Alu.add)
                scratch = sp.tile([P, ls], dt.float32, tag="scr",
                                  name=f"scr{t}")
                pairs = [(0, 0), (0, 1), (0, 2), (1, 1), (1, 2), (2, 2),
                         (3, 3)]
                for q, (i, j) in enumerate(pairs):
                    nc.vector.scalar_tensor_tensor(
                        out=scratch[:],
                        in0=Gf[:, i, :], scalar=1.0, in1=Gf[:, j, :],
                        op0=Alu.mult, op1=Alu.mult,
                        accum_out=PROD[:, q * GW + tl:q * GW + tl + 1])
                nc.vector.tensor_scalar(
                    out=CEN[:].rearrange("p (f t) -> p f t", t=GW)[:, :, tl],
                    in0=SUMS[:].rearrange("p (f t) -> p f t", t=GW)[:, 0:3, tl],
                    scalar1=RN[:, t:t + 1], scalar2=None, op0=Alu.mult)
                xc = xp.tile([P, 3 * ls], dt.float32, tag=f"xc{t % GW}",
                             name=f"xc{t}")
                for i in range(3):
                    nc.vector.tensor_scalar(
                        out=xc[:, i * ls:(i + 1) * ls],
                        in0=Gf[:, i, :],
                        scalar1=CEN[:, i * GW + tl:i * GW + tl + 1],
                        scalar2=None, op0=Alu.subtract)
                xcs.append(xc)

            def cluster_math(g):
                g0, g1 = g * GW, (g + 1) * GW
                SUMS, OH, PROD, CEN = SUMSg[g], OHg[g], PRODg[g], CENg[g]
                NVv, RNv = NV[:, g0:g1], RN[:, g0:g1]

                def tmp(tag, k=1):
                    return ret.tile([P, k * GW], dt.float32,
                                    tag=f"{tag}_g{g}", name=f"{tag}_g{g}")

                def sl(T, i):
                    return T[:, i * GW:(i + 1) * GW]

                def osl(T, i):
                    return T[:, i * NT + g0:i * NT + g1]

                A = tmp("A", 6)
                cmap = [(0, 0, 0), (1, 0, 1), (2, 0, 2), (3, 1, 1), (4, 1, 2),
                        (5, 2, 2)]
                SC1 = tmp("SC1")
                for q, i, j in cmap:
                    tt(Alu.mult, SC1[:], sl(CEN, i), sl(SUMS, j))
                    tt(Alu.subtract, sl(A, q), sl(PROD, q), SC1[:])

                VAR = tmp("VAR"); NM1 = tmp("NM1")
                tt(Alu.mult, osl(MEANV, 0), sl(SUMS, 3), RNv)
                tt(Alu.mult, VAR[:], osl(MEANV, 0), sl(SUMS, 3))
                tt(Alu.subtract, VAR[:], sl(PROD, 6), VAR[:])
                ts(NM1[:], NVv, 1.0, Alu.subtract)
                nc.vector.reciprocal(SC1[:], NM1[:])
                tt(Alu.mult, VAR[:], VAR[:], SC1[:])
                ts(VAR[:], VAR[:], 0.0, Alu.max)
                act(osl(STDV, 0), VAR[:], Act.Sqrt)

                BEST = tmp("BEST"); GT = tmp("GT"); KT = tmp("KT")

                Q = tmp("Q"); P1 = tmp("P1"); P2 = tmp("P2"); PP = tmp("PP")
                RP = tmp("RP"); DET = tmp("DET"); RR = tmp("RR"); SS = tmp("SS")
                AT = tmp("AT"); PHI = tmp("PHI")
                W0 = tmp("W0"); W1 = tmp("W1"); W2 = tmp("W2"); RW2 = tmp("RW2")
                DIRWT = tmp("DIRWT")
                NB = tmp("NB", 6)

                tt(Alu.add, Q[:], sl(A, 0), sl(A, 3))
                tt(Alu.add, Q[:], Q[:], sl(A, 5))
                ts(Q[:], Q[:], 1.0 / 3.0, Alu.mult)

                tt(Alu.mult, P1[:], sl(A, 1), sl(A, 1))
                tt(Alu.mult, SC1[:], sl(A, 2), sl(A, 2))
                tt(Alu.add, P1[:], P1[:], SC1[:])
                tt(Alu.mult, SC1[:], sl(A, 4), sl(A, 4))
                tt(Alu.add, P1[:], P1[:], SC1[:])

                BD = tmp("BD", 3)
                tt(Alu.subtract, sl(BD, 0), sl(A, 0), Q[:])
                tt(Alu.subtract, sl(BD, 1), sl(A, 3), Q[:])
                tt(Alu.subtract, sl(BD, 2), sl(A, 5), Q[:])
                tt(Alu.mult, P2[:], sl(BD, 0), sl(BD, 0))
                tt(Alu.mult, SC1[:], sl(BD, 1), sl(BD, 1))
                tt(Alu.add, P2[:], P2[:], SC1[:])
                tt(Alu.mult, SC1[:], sl(BD, 2), sl(BD, 2))
                tt(Alu.add, P2[:], P2[:], SC1[:])
                stt(P2[:], P1[:], 2.0, Alu.mult, Alu.add, P2[:])
                ts(PP[:], P2[:], 1.0 / 6.0, Alu.mult)
                act(PP[:], PP[:], Act.Sqrt)
                ts(SC1[:], PP[:], TINY, Alu.max)
                nc.vector.reciprocal(RP[:], SC1[:])

                tt(Alu.mult, sl(NB, 0), sl(BD, 0), RP[:])
                tt(Alu.mult, sl(NB, 1), sl(A, 1), RP[:])
                tt(Alu.mult, sl(NB, 2), sl(A, 2), RP[:])
                tt(Alu.mult, sl(NB, 3), sl(BD, 1), RP[:])
                tt(Alu.mult, sl(NB, 4), sl(A, 4), RP[:])
                tt(Alu.mult, sl(NB, 5), sl(BD, 2), RP[:])

                SC2 = tmp("SC2"); SC3 = tmp("SC3")
                tt(Alu.mult, SC1[:], sl(NB, 3), sl(NB, 5))
                tt(Alu.mult, SC2[:], sl(NB, 4), sl(NB, 4))
                tt(Alu.subtract, SC1[:], SC1[:], SC2[:])
                tt(Alu.mult, DET[:], sl(NB, 0), SC1[:])
                tt(Alu.mult, SC1[:], sl(NB, 1), sl(NB, 5))
                tt(Alu.mult, SC2[:], sl(NB, 4), sl(NB, 2))
                tt(Alu.subtract, SC1[:], SC1[:], SC2[:])
                tt(Alu.mult, SC1[:], sl(NB, 1), SC1[:])
                tt(Alu.subtract, DET[:], DET[:], SC1[:])
                tt(Alu.mult, SC1[:], sl(NB, 1), sl(NB, 4))
                tt(Alu.mult, SC2[:], sl(NB, 3), sl(NB, 2))
                tt(Alu.subtract, SC1[:], SC1[:], SC2[:])
                tt(Alu.mult, SC1[:], sl(NB, 2), SC1[:])
                tt(Alu.add, DET[:], DET[:], SC1[:])

                ts(RR[:], DET[:], 0.5, Alu.mult)
                ts(RR[:], RR[:], -1.0, Alu.max)
                ts(RR[:], RR[:], 1.0, Alu.min)
                tt(Alu.mult, SS[:], RR[:], RR[:])
                nc.vector.tensor_scalar(out=SS[:], in0=SS[:], scalar1=-1.0,
                                        scalar2=1.0, op0=Alu.mult, op1=Alu.add)
                ts(SS[:], SS[:], 0.0, Alu.max)
                act(SS[:], SS[:], Act.Sqrt)
                UA = tmp("UA"); UB = tmp("UB")
                ts(SC1[:], RR[:], -1.0, Alu.mult)
                tt(Alu.max, SC1[:], SC1[:], RR[:])
                ts(SS[:], SS[:], TINY, Alu.max)
                nc.vector.reciprocal(SC2[:], SS[:])
                tt(Alu.mult, UA[:], SC1[:], SC2[:])
                ts(SC1[:], UA[:], TINY, Alu.max)
                nc.vector.reciprocal(UB[:], SC1[:])
                tt(Alu.min, SC2[:], UA[:], UB[:])
                act(SC2[:], SC2[:], Act.Arctan)
                ts(SC1[:], UA[:], 1.0, Alu.is_gt)
                nc.vector.tensor_scalar(out=SC3[:], in0=SC2[:], scalar1=-2.0,
                                        scalar2=_PI / 2.0, op0=Alu.mult,
                                        op1=Alu.add)
                tt(Alu.mult, SC3[:], SC3[:], SC1[:])
                tt(Alu.add, SC2[:], SC2[:], SC3[:])
                ts(SC3[:], RR[:], 0.0, Alu.is_lt)
                nc.vector.tensor_scalar(out=SC3[:], in0=SC3[:], scalar1=-2.0,
                                        scalar2=1.0, op0=Alu.mult, op1=Alu.add)
                tt(Alu.mult, AT[:], SC2[:], SC3[:])
                nc.vector.tensor_scalar(out=PHI[:], in0=AT[:],
                                        scalar1=-1.0 / 3.0,
                                        scalar2=_PI / 6.0 + _PI / 2.0,
                                        op0=Alu.mult, op1=Alu.add)
                act(SC1[:], PHI[:], Act.Sin)
                tt(Alu.mult, SC1[:], SC1[:], PP[:])
                stt(W2[:], SC1[:], 2.0, Alu.mult, Alu.add, Q[:])
                nc.vector.tensor_scalar(out=PHI[:], in0=AT[:],
                                        scalar1=-1.0 / 3.0,
                                        scalar2=_PI / 6.0 + _PI / 6.0,
                                        op0=Alu.mult, op1=Alu.add)
                act(SC1[:], PHI[:], Act.Sin)
                tt(Alu.mult, SC1[:], SC1[:], PP[:])
                stt(W0[:], SC1[:], -2.0, Alu.mult, Alu.add, Q[:])
                ts(SC1[:], Q[:], 3.0, Alu.mult)
                tt(Alu.subtract, W1[:], SC1[:], W0[:])
                tt(Alu.subtract, W1[:], W1[:], W2[:])

                ts(SC1[:], W2[:], TINY, Alu.max)
                nc.vector.reciprocal(RW2[:], SC1[:])
                tt(Alu.mult, DIRWT[:], W1[:], RW2[:])
                nc.vector.tensor_scalar(out=DIRWT[:], in0=DIRWT[:],
                                        scalar1=-1.0, scalar2=1.0,
                                        op0=Alu.mult, op1=Alu.add)
                for q in range(6):
                    tt(Alu.mult, osl(B6, q), sl(A, q), RW2[:])

                CD = tmp("CD", 3)
                DD = tmp("DD", 3)
                for qi, ai in enumerate((0, 3, 5)):
                    tt(Alu.subtract, sl(CD, qi), sl(A, ai), W0[:])
                    tt(Alu.subtract, sl(DD, qi), sl(A, ai), W1[:])
                M9 = tmp("M9", 9)

                def mcol(colq, dv):
                    crow = [(sl(CD, 0), sl(A, 1), sl(A, 2)),
                            (sl(A, 1), sl(CD, 1), sl(A, 4)),
                            (sl(A, 2), sl(A, 4), sl(CD, 2))]
                    for r in range(3):
                        a0, a1, a2 = crow[r]
                        tt(Alu.mult, SC1[:], a0, dv[0])
                        tt(Alu.mult, SC2[:], a1, dv[1])
                        tt(Alu.add, SC1[:], SC1[:], SC2[:])
                        tt(Alu.mult, SC2[:], a2, dv[2])
                        tt(Alu.add, sl(M9, colq * 3 + r), SC1[:], SC2[:])

                mcol(0, (sl(DD, 0), sl(A, 1), sl(A, 2)))
                mcol(1, (sl(A, 1), sl(DD, 1), sl(A, 4)))
                mcol(2, (sl(A, 2), sl(A, 4), sl(DD, 2)))

                CN = tmp("CN", 3)
                for j in range(3):
                    tt(Alu.mult, sl(CN, j), sl(M9, j * 3), sl(M9, j * 3))
                    tt(Alu.mult, SC1[:], sl(M9, j * 3 + 1), sl(M9, j * 3 + 1))
                    tt(Alu.add, sl(CN, j), sl(CN, j), SC1[:])
                    tt(Alu.mult, SC1[:], sl(M9, j * 3 + 2), sl(M9, j * 3 + 2))
                    tt(Alu.add, sl(CN, j), sl(CN, j), SC1[:])
                NBEST = tmp("NBEST")
                for i in range(3):
                    nc.vector.tensor_copy(out=osl(V3, i), in_=sl(M9, i))
                nc.vector.tensor_copy(out=NBEST[:], in_=sl(CN, 0))
                for j in (1, 2):
                    tt(Alu.is_gt, GT[:], sl(CN, j), NBEST[:])
                    for i in range(3):
                        tt(Alu.subtract, SC1[:], sl(M9, j * 3 + i), osl(V3, i))
                        tt(Alu.mult, SC1[:], SC1[:], GT[:])
                        tt(Alu.add, osl(V3, i), osl(V3, i), SC1[:])
                    tt(Alu.max, NBEST[:], NBEST[:], sl(CN, j))
                ts(SC1[:], NBEST[:], 1e-37, Alu.max)
                act(SC2[:], SC1[:], Act.Sqrt)
                nc.vector.reciprocal(SC2[:], SC2[:])
                for i in range(3):
                    tt(Alu.mult, osl(V3, i), osl(V3, i), SC2[:])

                tt(Alu.subtract, BEST[:], NVv, sl(OH, 0))
                for k in (1, 2, 3):
                    tt(Alu.subtract, BEST[:], BEST[:], sl(OH, k))
                nc.vector.memset(osl(MODE, 0), 0.0)
                for k in range(1, 5):
                    ck = sl(OH, k - 1)
                    tt(Alu.is_gt, GT[:], ck, BEST[:])
                    nc.vector.tensor_scalar(out=KT[:], in0=osl(MODE, 0),
                                            scalar1=-1.0, scalar2=float(k),
                                            op0=Alu.mult, op1=Alu.add)
                    tt(Alu.mult, KT[:], KT[:], GT[:])
                    tt(Alu.add, osl(MODE, 0), osl(MODE, 0), KT[:])
                    tt(Alu.max, BEST[:], BEST[:], ck)
                return DIRWT

            def pass_b(t):
                ls = F * int(Ob[t])
                xc = xcs[t]
                g = t // GW
                xcx = xc[:, 0:ls]; xcy = xc[:, ls:2 * ls]
                xcz = xc[:, 2 * ls:3 * ls]
                T = sp.tile([P, ls], dt.float32, tag="T", name=f"T{t}")
                S2 = sp.tile([P, ls], dt.float32, tag="S2", name=f"S2_{t}")
                S2b = sp.tile([P, ls], dt.float32, tag="S2b", name=f"S2b{t}")
                R = sp.tile([P, ls], dt.float32, tag="R", name=f"R{t}")
                nc.vector.tensor_scalar(out=T[:], in0=xcx,
                                        scalar1=V3[:, 0 * NT + t:0 * NT + t + 1],
                                        scalar2=None, op0=Alu.mult)
                stt(T[:], xcy, V3[:, 1 * NT + t:1 * NT + t + 1],
                    Alu.mult, Alu.add, T[:])
                stt(T[:], xcz, V3[:, 2 * NT + t:2 * NT + t + 1],
                    Alu.mult, Alu.add, T[:])
                stt(S2[:], xcx, 1.0, Alu.mult, Alu.mult, xcx)
                stt(S2b[:], xcy, 1.0, Alu.mult, Alu.mult, xcy)
                tt(Alu.add, S2[:], S2[:], S2b[:])
                stt(S2b[:], xcz, 1.0, Alu.mult, Alu.mult, xcz)
                tt(Alu.add, S2[:], S2[:], S2b[:])
                stt(S2b[:], T[:], 1.0, Alu.mult, Alu.mult, T[:])
                tt(Alu.subtract, S2[:], S2[:], S2b[:])
                ts(S2[:], S2[:], 0.0, Alu.max)
                act(R[:], S2[:], Act.Sqrt)
                stt(S2b[:], T[:], 1.0, Alu.mult, Alu.mult, R[:],
                    accum=SCRAWg[g][:, t % GW:t % GW + 1])

            def sign_phase(g, DIRWT):
                g0, g1 = g * GW, (g + 1) * GW
                CEN = CENg[g]; SCRAW = SCRAWg[g]
                NVv = NV[:, g0:g1]

                def tmp(tag, k=1):
                    return ret.tile([P, k * GW], dt.float32,
                                    tag=f"{tag}_g{g}", name=f"{tag}_g{g}")

                def sl(T, i):
                    return T[:, i * GW:(i + 1) * GW]

                def osl(T, i):
                    return T[:, i * NT + g0:i * NT + g1]

                T0 = tmp("T0"); CC = tmp("CC"); R0 = tmp("R0")
                SCV = tmp("SCV"); FAC = tmp("FAC"); SC9 = tmp("SC9")
                GT9 = tmp("GT9"); NPAD = tmp("NPAD")
                tt(Alu.mult, T0[:], sl(CEN, 0), osl(V3, 0))
                tt(Alu.mult, SC9[:], sl(CEN, 1), osl(V3, 1))
                tt(Alu.add, T0[:], T0[:], SC9[:])
                tt(Alu.mult, SC9[:], sl(CEN, 2), osl(V3, 2))
                tt(Alu.add, T0[:], T0[:], SC9[:])
                ts(T0[:], T0[:], -1.0, Alu.mult)
                tt(Alu.mult, CC[:], sl(CEN, 0), sl(CEN, 0))
                tt(Alu.mult, SC9[:], sl(CEN, 1), sl(CEN, 1))
                tt(Alu.add, CC[:], CC[:], SC9[:])
                tt(Alu.mult, SC9[:], sl(CEN, 2), sl(CEN, 2))
                tt(Alu.add, CC[:], CC[:], SC9[:])
                tt(Alu.mult, SC9[:], T0[:], T0[:])
                tt(Alu.subtract, R0[:], CC[:], SC9[:])
                ts(R0[:], R0[:], 0.0, Alu.max)
                act(R0[:], R0[:], Act.Sqrt)
                for t in range(g0, g1):
                    nc.vector.tensor_scalar(
                        out=NPAD[:, t - g0:t - g0 + 1],
                        in0=NV[:, t:t + 1], scalar1=-1.0,
                        scalar2=float(F * int(Ob[t])), op0=Alu.mult,
                        op1=Alu.add)
                tt(Alu.mult, SC9[:], T0[:], R0[:])
                tt(Alu.mult, SC9[:], SC9[:], NPAD[:])
                tt(Alu.subtract, SCV[:], SCRAW[:], SC9[:])
                ts(GT9[:], SCV[:], 0.0, Alu.is_lt)
                nc.vector.tensor_scalar(out=GT9[:], in0=GT9[:], scalar1=-2.0,
                                        scalar2=1.0, op0=Alu.mult, op1=Alu.add)
                tt(Alu.mult, FAC[:], DIRWT[:], GT9[:])
                for i in range(3):
                    tt(Alu.mult, osl(V3, i), osl(V3, i), FAC[:])
                for j, pl in [(0, sl(CEN, 0)), (1, sl(CEN, 1)), (2, sl(CEN, 2)),
                              (3, osl(B6, 0)), (4, osl(B6, 1)), (5, osl(B6, 2)),
                              (6, osl(B6, 1)), (7, osl(B6, 3)), (8, osl(B6, 4)),
                              (9, osl(B6, 2)), (10, osl(B6, 4)),
                              (11, osl(B6, 5)),
                              (12, osl(V3, 0)), (13, osl(V3, 1)),
                              (14, osl(V3, 2)),
                              (15, NVv), (16, osl(MEANV, 0)),
                              (17, osl(STDV, 0)),
                              (18, osl(MODE, 0))]:
                    nc.sync.dma_start(out=res[:, j * NT + g0:j * NT + g1],
                                      in_=pl)

            for g in range(NG):
                for t in range(g * GW, (g + 1) * GW):
                    gather_and_pass_a(t)
                DIRWT = cluster_math(g)
                for t in range(g * GW, (g + 1) * GW):
                    pass_b(t)
                sign_phase(g, DIRWT)

    nc.compile()
    return nc


_cache = {}
_last = None


def kernel(data, clust_idx, clust_len):
    global N, C, L, NT
    data = np.asarray(data)
    clust_idx = np.asarray(clust_idx)
    N = int(data.shape[0])
    C, L = int(clust_idx.shape[0]), int(clust_idx.shape[1])
    assert C % (P * N_CORES) == 0, \
        f"cluster count {C} not divisible by {P * N_CORES}"
    NT = C // (P * N_CORES)
    table, idx_blobs, clen_blobs, nvecs, Ob, S, ids = _host_prep(
        data, clust_idx, clust_len)

    key = tuple(int(x) for x in Ob)
    if key not in _cache:
        _cache[key] = _build_program(Ob, S)
    nc = _cache[key]

    from concourse.bass_utils import run_bass_kernel_spmd
    in_maps = [{"table": table, "idx": idx_blobs[c], "clen": clen_blobs[c],
                "nvec": nvecs[c]}
               for c in range(N_CORES)]
    global _last
    _last = (nc, in_maps)
    res = run_bass_kernel_spmd(nc, in_maps, list(range(N_CORES)))

    out = np.zeros((C, 19), dtype=f32)
    for core in range(N_CORES):
        r = res.results[core]["res"].reshape(P, 19, NT)
        for t in range(NT):
            out[ids[core, t]] = r[:, :, t]
    return out


# revision 6
# speedup vs baseline: 1.3806x; 1.0005x over previous
"""Trainium2 Bass kernel for nn_ClustGeoNodeEncoder (segment_reduce).

Strategy (data-parallel over the cluster axis, per the sharding hint):
  - Host packs the voxel table as [N+4, 8] f32 rows: x, y, z, value,
    onehot(sem==1..4); the table rows are PERMUTED so that each cluster's
    members tend to sit at consecutive positions (greedy claim pass), and
    the last 4 rows are zeros (targets of padded offsets).
  - Each cluster's member list is grouped into chains of up to 4
    consecutive permuted positions.  One indirect-DMA offset fetches a
    whole chain (payload 4 rows = 128B), cutting the dominant cost — the
    ~1.4us INDIRECT1D instruction per 128 offsets — by ~30%.
  - Chain slots beyond the chain length are zeroed on-device with
    (chainlen > k) masks, making them exact zero rows, so the moment sums
    need no other masking and the padded-slot closed-form correction in
    the orientation pass applies unchanged.
  - Clusters are sorted by offset count and dealt round-robin to the 8
    cores (SPMD): 32 tiles x 128 clusters per core, tile t padded to
    Ob[t] = max offsets in its rank range.
  - Pass A (per tile): raw sums / second moments / value stats / semantic
    counts via fused multiply-accumulate and strided tensor_reduce;
    centered coordinates are retained in SBUF (per-group recycled pool).
  - Batched per-cluster math on [128, GW] tiles: closed-form symmetric
    3x3 eigenvalues (trig method), principal eigenvector via the spectral
    projector, B = A / w2, dirwt = 1 - w1/w2, mode via argmax scan.
  - Pass B (per tile): orientation statistic from the retained centered
    coords; zero slots contribute a closed-form correction term.
"""

import sys

for _p in ("/opt/trn_rl_repo",):
    if _p not in sys.path:
        sys.path.insert(0, _p)

import numpy as np

N = 2_000_000
C = 32768
L = 256
N_CORES = 8
P = 128
NT = C // (P * N_CORES)  # 32 tiles per core
F = 4                    # chain length (rows per indirect offset)
f32 = np.float32

_PI = float(np.pi)


def _host_prep(data, clust_idx, clust_len):
    data = np.asarray(data, dtype=f32)
    clust_idx = np.asarray(clust_idx).astype(np.int64)
    lens = np.asarray(clust_len).astype(np.int64)

    base = np.zeros((N, 8), dtype=f32)
    base[:, 0:3] = data[:, 0:3]
    base[:, 3] = data[:, 4]
    sem = data[:, 5].astype(np.int32)
    for k in range(1, 5):
        base[:, 3 + k] = (sem == k)

    # --- claim pass: greedy per-cluster runs -> table permutation pos[] ---
    rng = np.random.default_rng(0)
    order_c = rng.permutation(C)
    claimed = np.zeros(N, bool)
    pos = np.full(N, -1, np.int64)
    nxt = 0
    for c in order_c:
        rows = np.unique(clust_idx[c, :lens[c]])
        un = rows[~claimed[rows]]
        claimed[un] = True
        pos[un] = nxt + np.arange(len(un))
        nxt += len(un)
    rest = np.where(~claimed)[0]
    pos[rest] = nxt + np.arange(len(rest))

    table = np.zeros((N + F, 8), dtype=f32)
    table[pos] = base

    # --- chain grouping per cluster ---
    starts_c = []
    clens_c = []
    o_c = np.zeros(C, np.int64)
    for c in range(C):
        q = np.sort(pos[clust_idx[c, :lens[c]]])
        n = len(q)
        st = []
        cl = []
        i = 0
        while i < n:
            j = i + 1
            while j < n and j - i < F and q[j] == q[i] + (j - i):
                j += 1
            st.append(q[i])
            cl.append(j - i)
            i = j
        starts_c.append(np.asarray(st, np.int32))
        clens_c.append(np.asarray(cl, f32))
        o_c[c] = len(st)

    # --- tile packing: sort by offset count (descending, so the last
    # processed group is the smallest -> minimal DVE tail), deal round-robin
    order = np.argsort(-o_c, kind="stable")
    Ob = np.zeros(NT, dtype=np.int64)
    for t in range(NT):
        Ob[t] = o_c[order[t * P * N_CORES:(t + 1) * P * N_CORES]].max()
    S = int(Ob.sum())

    idx_blobs = np.full((N_CORES, P, S), N, dtype=np.int32)
    clen_blobs = np.full((N_CORES, P, S), float(F), dtype=f32)
    nvecs = np.zeros((N_CORES, P, NT), dtype=f32)
    ids = np.zeros((N_CORES, NT, P), dtype=np.int64)
    off = 0
    for t in range(NT):
        tb = t * P * N_CORES
        for core in range(N_CORES):
            sel = order[tb + core + N_CORES * np.arange(P)]
            ids[core, t] = sel
            nvecs[core, :, t] = lens[sel]
            for p in range(P):
                c = sel[p]
                o = len(starts_c[c])
                idx_blobs[core, p, off:off + o] = starts_c[c]
                clen_blobs[core, p, off:off + o] = clens_c[c]
        off += Ob[t]
    return table, idx_blobs, clen_blobs, nvecs, Ob, S, ids


def _build_program(Ob, S):
    import concourse.bass as bass
    import concourse.bacc as bacc
    import concourse.mybir as mybir
    from concourse.tile import TileContext

    dt = mybir.dt
    Alu = mybir.AluOpType
    Act = mybir.ActivationFunctionType

    nc = bacc.Bacc("TRN2", target_bir_lowering=False, debug=False,
                   enable_asserts=False)
    table = nc.dram_tensor("table", [N + F, 8], dt.float32,
                           kind="ExternalInput")
    idx = nc.dram_tensor("idx", [P, S], dt.int32, kind="ExternalInput")
    clen_d = nc.dram_tensor("clen", [P, S], dt.float32, kind="ExternalInput")
    nvec_d = nc.dram_tensor("nvec", [P, NT], dt.float32, kind="ExternalInput")
    res = nc.dram_tensor("res", [P, 19 * NT], dt.float32,
                         kind="ExternalOutput")

    TINY = 1e-30

    NG = 8 if NT % 8 == 0 else (2 if NT % 2 == 0 else 1)
    GW = NT // NG

    with TileContext(nc) as tc:
        with tc.tile_pool(name="ret", bufs=1) as ret, \
             tc.tile_pool(name="gp", bufs=2) as gp, \
             tc.tile_pool(name="xp", bufs=1) as xp, \
             tc.tile_pool(name="sp", bufs=1) as sp:

            def full_tile(tag, k=1):
                return ret.tile([P, k * NT], dt.float32, tag=tag, name=tag)

            NV = full_tile("NV")
            RN = full_tile("RN")

            def gw_tile(tag, g, k=1):
                return ret.tile([P, k * GW], dt.float32, tag=f"{tag}_g{g}",
                                name=f"{tag}_g{g}")
            SUMSg = [gw_tile("SUMS", g, 4) for g in range(NG)]
            OHg = [gw_tile("OH", g, 4) for g in range(NG)]
            PRODg = [gw_tile("PROD", g, 7) for g in range(NG)]
            CENg = [gw_tile("CEN", g, 3) for g in range(NG)]
            SCRAWg = [gw_tile("SCRAW", g) for g in range(NG)]
            MEANV = full_tile("MEANV"); STDV = full_tile("STDV")
            MODE = full_tile("MODE")
            B6 = full_tile("B6", 6)
            V3 = full_tile("V3", 3)

            # whole-run preloads: offsets + chain lengths + cluster sizes
            ITALL = ret.tile([P, S], dt.int32, tag="ITALL", name="ITALL")
            CLALL = ret.tile([P, S], dt.float32, tag="CLALL", name="CLALL")
            ob0 = int(Ob[0])
            nc.sync.dma_start(out=ITALL[:, 0:ob0], in_=idx[:, 0:ob0])
            nc.sync.dma_start(out=ITALL[:, ob0:S], in_=idx[:, ob0:S])
            nc.sync.dma_start(out=CLALL[:], in_=clen_d[:, :])
            nc.sync.dma_start(out=NV[:], in_=nvec_d[:, :])
            nc.vector.reciprocal(RN[:], NV[:])

            def tt(op, out, a, b):
                nc.vector.tensor_tensor(out=out, in0=a, in1=b, op=op)

            def ts(out, in0, s, op):
                nc.vector.tensor_scalar(out=out, in0=in0, scalar1=s,
                                        scalar2=None, op0=op)

            def stt(out, in0, s, op0, op1, in1, accum=None):
                nc.vector.scalar_tensor_tensor(out=out, in0=in0, scalar=s,
                                               in1=in1, op0=op0, op1=op1,
                                               accum_out=accum)

            def act(out, in_, func, bias=0.0, scale=1.0):
                nc.scalar.activation(out, in_, func, bias=bias, scale=scale)

            xcs = []
            offs = []
            off = 0
            for t in range(NT):
                offs.append(off)
                off += int(Ob[t])

            def gather_and_pass_a(t):
                g, tl = t // GW, t % GW
                ob = int(Ob[t])
                ls = F * ob          # slot count for this tile
                SUMS, OH, PROD, CEN = SUMSg[g], OHg[g], PRODg[g], CENg[g]
                G = gp.tile([P, ls * 8], dt.float32, tag="G", name=f"G{t}")
                for l in range(ob):
                    nc.gpsimd.indirect_dma_start(
                        out=G[:, l * F * 8:(l + 1) * F * 8],
                        out_offset=None,
                        in_=table[:, :],
                        in_offset=bass.IndirectOffsetOnAxis(
                            ap=ITALL[:, offs[t] + l:offs[t] + l + 1], axis=0),
                    )
                # zero chain slots beyond chainlen: slot k valid iff cl > k
                Gs = G[:].rearrange("p (o w) -> p o w", w=F * 8)
                MK = sp.tile([P, ob], dt.float32, tag="MK", name=f"MK{t}")
                for k in range(1, F):
                    ts(MK[:], CLALL[:, offs[t]:offs[t] + ob], float(k),
                       Alu.is_gt)
                    for f in range(8):
                        tt(Alu.mult, Gs[:, :, k * 8 + f], Gs[:, :, k * 8 + f],
                           MK[:])
                Gf = G[:].rearrange("p (l f) -> p f l", f=8)
                nc.vector.tensor_reduce(
                    out=SUMS[:].rearrange("p (f t) -> p f t", t=GW)[:, :, tl],
                    in_=Gf[:, 0:4, :], axis=mybir.AxisListType.X, op=Alu.add)
                nc.vector.tensor_reduce(
                    out=OH[:].rearrange("p (f t) -> p f t", t=GW)[:, :, tl],
                    in_=Gf[:, 4:8, :], axis=mybir.AxisListType.X, op=Alu.add)
                scratch = sp.tile([P, ls], dt.float32, tag="scr",
                                  name=f"scr{t}")
                pairs = [(0, 0), (0, 1), (0, 2), (1, 1), (1, 2), (2, 2),
                         (3, 3)]
                for q, (i, j) in enumerate(pairs):
                    nc.vector.scalar_tensor_tensor(
                        out=scratch[:],
                        in0=Gf[:, i, :], scalar=1.0, in1=Gf[:, j, :],
                        op0=Alu.mult, op1=Alu.mult,
                        accum_out=PROD[:, q * GW + tl:q * GW + tl + 1])
                nc.vector.tensor_scalar(
                    out=CEN[:].rearrange("p (f t) -> p f t", t=GW)[:, :, tl],
                    in0=SUMS[:].rearrange("p (f t) -> p f t", t=GW)[:, 0:3, tl],
                    scalar1=RN[:, t:t + 1], scalar2=None, op0=Alu.mult)
                xc = xp.tile([P, 3 * ls], dt.float32, tag=f"xc{t % GW}",
                             name=f"xc{t}")
                for i in range(3):
                    nc.vector.tensor_scalar(
                        out=xc[:, i * ls:(i + 1) * ls],
                        in0=Gf[:, i, :],
                        scalar1=CEN[:, i * GW + tl:i * GW + tl + 1],
                        scalar2=None, op0=Alu.subtract)
                xcs.append(xc)

            def cluster_math(g):
                g0, g1 = g * GW, (g + 1) * GW
                SUMS, OH, PROD, CEN = SUMSg[g], OHg[g], PRODg[g], CENg[g]
                NVv, RNv = NV[:, g0:g1], RN[:, g0:g1]

                def tmp(tag, k=1):
                    return ret.tile([P, k * GW], dt.float32,
                                    tag=f"{tag}_g{g}", name=f"{tag}_g{g}")

                def sl(T, i):
                    return T[:, i * GW:(i + 1) * GW]

                def osl(T, i):
                    return T[:, i * NT + g0:i * NT + g1]

                A = tmp("A", 6)
                cmap = [(0, 0, 0), (1, 0, 1), (2, 0, 2), (3, 1, 1), (4, 1, 2),
                        (5, 2, 2)]
                SC1 = tmp("SC1")
                for q, i, j in cmap:
                    tt(Alu.mult, SC1[:], sl(CEN, i), sl(SUMS, j))
                    tt(Alu.subtract, sl(A, q), sl(PROD, q), SC1[:])

                VAR = tmp("VAR"); NM1 = tmp("NM1")
                tt(Alu.mult, osl(MEANV, 0), sl(SUMS, 3), RNv)
                tt(Alu.mult, VAR[:], osl(MEANV, 0), sl(SUMS, 3))
                tt(Alu.subtract, VAR[:], sl(PROD, 6), VAR[:])
                ts(NM1[:], NVv, 1.0, Alu.subtract)
                nc.vector.reciprocal(SC1[:], NM1[:])
                tt(Alu.mult, VAR[:], VAR[:], SC1[:])
                ts(VAR[:], VAR[:], 0.0, Alu.max)
                act(osl(STDV, 0), VAR[:], Act.Sqrt)

                BEST = tmp("BEST"); GT = tmp("GT"); KT = tmp("KT")

                Q = tmp("Q"); P1 = tmp("P1"); P2 = tmp("P2"); PP = tmp("PP")
                RP = tmp("RP"); DET = tmp("DET"); RR = tmp("RR"); SS = tmp("SS")
                AT = tmp("AT"); PHI = tmp("PHI")
                W0 = tmp("W0"); W1 = tmp("W1"); W2 = tmp("W2"); RW2 = tmp("RW2")
                DIRWT = tmp("DIRWT")
                NB = tmp("NB", 6)

                tt(Alu.add, Q[:], sl(A, 0), sl(A, 3))
                tt(Alu.add, Q[:], Q[:], sl(A, 5))
                ts(Q[:], Q[:], 1.0 / 3.0, Alu.mult)

                tt(Alu.mult, P1[:], sl(A, 1), sl(A, 1))
                tt(Alu.mult, SC1[:], sl(A, 2), sl(A, 2))
                tt(Alu.add, P1[:], P1[:], SC1[:])
                tt(Alu.mult, SC1[:], sl(A, 4), sl(A, 4))
                tt(Alu.add, P1[:], P1[:], SC1[:])

                BD = tmp("BD", 3)
                tt(Alu.subtract, sl(BD, 0), sl(A, 0), Q[:])
                tt(Alu.subtract, sl(BD, 1), sl(A, 3), Q[:])
                tt(Alu.subtract, sl(BD, 2), sl(A, 5), Q[:])
                tt(Alu.mult, P2[:], sl(BD, 0), sl(BD, 0))
                tt(Alu.mult, SC1[:], sl(BD, 1), sl(BD, 1))
                tt(Alu.add, P2[:], P2[:], SC1[:])
                tt(Alu.mult, SC1[:], sl(BD, 2), sl(BD, 2))
                tt(Alu.add, P2[:], P2[:], SC1[:])
                stt(P2[:], P1[:], 2.0, Alu.mult, Alu.add, P2[:])
                ts(PP[:], P2[:], 1.0 / 6.0, Alu.mult)
                act(PP[:], PP[:], Act.Sqrt)
                ts(SC1[:], PP[:], TINY, Alu.max)
                nc.vector.reciprocal(RP[:], SC1[:])

                tt(Alu.mult, sl(NB, 0), sl(BD, 0), RP[:])
                tt(Alu.mult, sl(NB, 1), sl(A, 1), RP[:])
                tt(Alu.mult, sl(NB, 2), sl(A, 2), RP[:])
                tt(Alu.mult, sl(NB, 3), sl(BD, 1), RP[:])
                tt(Alu.mult, sl(NB, 4), sl(A, 4), RP[:])
                tt(Alu.mult, sl(NB, 5), sl(BD, 2), RP[:])

                SC2 = tmp("SC2"); SC3 = tmp("SC3")
                tt(Alu.mult, SC1[:], sl(NB, 3), sl(NB, 5))
                tt(Alu.mult, SC2[:], sl(NB, 4), sl(NB, 4))
                tt(Alu.subtract, SC1[:], SC1[:], SC2[:])
                tt(Alu.mult, DET[:], sl(NB, 0), SC1[:])
                tt(Alu.mult, SC1[:], sl(NB, 1), sl(NB, 5))
                tt(Alu.mult, SC2[:], sl(NB, 4), sl(NB, 2))
                tt(Alu.subtract, SC1[:], SC1[:], SC2[:])
                tt(Alu.mult, SC1[:], sl(NB, 1), SC1[:])
                tt(Alu.subtract, DET[:], DET[:], SC1[:])
                tt(Alu.mult, SC1[:], sl(NB, 1), sl(NB, 4))
                tt(Alu.mult, SC2[:], sl(NB, 3), sl(NB, 2))
                tt(Alu.subtract, SC1[:], SC1[:], SC2[:])
                tt(Alu.mult, SC1[:], sl(NB, 2), SC1[:])
                tt(Alu.add, DET[:], DET[:], SC1[:])

                ts(RR[:], DET[:], 0.5, Alu.mult)
                ts(RR[:], RR[:], -1.0, Alu.max)
                ts(RR[:], RR[:], 1.0, Alu.min)
                tt(Alu.mult, SS[:], RR[:], RR[:])
                nc.vector.tensor_scalar(out=SS[:], in0=SS[:], scalar1=-1.0,
                                        scalar2=1.0, op0=Alu.mult, op1=Alu.add)
                ts(SS[:], SS[:], 0.0, Alu.max)
                act(SS[:], SS[:], Act.Sqrt)
                UA = tmp("UA"); UB = tmp("UB")
                ts(SC1[:], RR[:], -1.0, Alu.mult)
                tt(Alu.max, SC1[:], SC1[:], RR[:])
                ts(SS[:], SS[:], TINY, Alu.max)
                nc.vector.reciprocal(SC2[:], SS[:])
                tt(Alu.mult, UA[:], SC1[:], SC2[:])
                ts(SC1[:], UA[:], TINY, Alu.max)
                nc.vector.reciprocal(UB[:], SC1[:])
                tt(Alu.min, SC2[:], UA[:], UB[:])
                act(SC2[:], SC2[:], Act.Arctan)
                ts(SC1[:], UA[:], 1.0, Alu.is_gt)
                nc.vector.tensor_scalar(out=SC3[:], in0=SC2[:], scalar1=-2.0,
                                        scalar2=_PI / 2.0, op0=Alu.mult,
                                        op1=Alu.add)
                tt(Alu.mult, SC3[:], SC3[:], SC1[:])
                tt(Alu.add, SC2[:], SC2[:], SC3[:])
                ts(SC3[:], RR[:], 0.0, Alu.is_lt)
                nc.vector.tensor_scalar(out=SC3[:], in0=SC3[:], scalar1=-2.0,
                                        scalar2=1.0, op0=Alu.mult, op1=Alu.add)
                tt(Alu.mult, AT[:], SC2[:], SC3[:])
                nc.vector.tensor_scalar(out=PHI[:], in0=AT[:],
                                        scalar1=-1.0 / 3.0,
                                        scalar2=_PI / 6.0 + _PI / 2.0,
                                        op0=Alu.mult, op1=Alu.add)
                act(SC1[:], PHI[:], Act.Sin)
                tt(Alu.mult, SC1[:], SC1[:], PP[:])
                stt(W2[:], SC1[:], 2.0, Alu.mult, Alu.add, Q[:])
                nc.vector.tensor_scalar(out=PHI[:], in0=AT[:],
                                        scalar1=-1.0 / 3.0,
                                        scalar2=_PI / 6.0 + _PI / 6.0,
                                        op0=Alu.mult, op1=Alu.add)
                act(SC1[:], PHI[:], Act.Sin)
                tt(Alu.mult, SC1[:], SC1[:], PP[:])
                stt(W0[:], SC1[:], -2.0, Alu.mult, Alu.add, Q[:])
                ts(SC1[:], Q[:], 3.0, Alu.mult)
                tt(Alu.subtract, W1[:], SC1[:], W0[:])
                tt(Alu.subtract, W1[:], W1[:], W2[:])

                ts(SC1[:], W2[:], TINY, Alu.max)
                nc.vector.reciprocal(RW2[:], SC1[:])
                tt(Alu.mult, DIRWT[:], W1[:], RW2[:])
                nc.vector.tensor_scalar(out=DIRWT[:], in0=DIRWT[:],
                                        scalar1=-1.0, scalar2=1.0,
                                        op0=Alu.mult, op1=Alu.add)
                for q in range(6):
                    tt(Alu.mult, osl(B6, q), sl(A, q), RW2[:])

                CD = tmp("CD", 3)
                DD = tmp("DD", 3)
                for qi, ai in enumerate((0, 3, 5)):
                    tt(Alu.subtract, sl(CD, qi), sl(A, ai), W0[:])
                    tt(Alu.subtract, sl(DD, qi), sl(A, ai), W1[:])
                M9 = tmp("M9", 9)

                def mcol(colq, dv):
                    crow = [(sl(CD, 0), sl(A, 1), sl(A, 2)),
                            (sl(A, 1), sl(CD, 1), sl(A, 4)),
                            (sl(A, 2), sl(A, 4), sl(CD, 2))]
                    for r in range(3):
                        a0, a1, a2 = crow[r]
                        tt(Alu.mult, SC1[:], a0, dv[0])
                        tt(Alu.mult, SC2[:], a1, dv[1])
                        tt(Alu.add, SC1[:], SC1[:], SC2[:])
                        tt(Alu.mult, SC2[:], a2, dv[2])
                        tt(Alu.add, sl(M9, colq * 3 + r), SC1[:], SC2[:])

                mcol(0, (sl(DD, 0), sl(A, 1), sl(A, 2)))
                mcol(1, (sl(A, 1), sl(DD, 1), sl(A, 4)))
                mcol(2, (sl(A, 2), sl(A, 4), sl(DD, 2)))

                CN = tmp("CN", 3)
                for j in range(3):
                    tt(Alu.mult, sl(CN, j), sl(M9, j * 3), sl(M9, j * 3))
                    tt(Alu.mult, SC1[:], sl(M9, j * 3 + 1), sl(M9, j * 3 + 1))
                    tt(Alu.add, sl(CN, j), sl(CN, j), SC1[:])
                    tt(Alu.mult, SC1[:], sl(M9, j * 3 + 2), sl(M9, j * 3 + 2))
                    tt(Alu.add, sl(CN, j), sl(CN, j), SC1[:])
                NBEST = tmp("NBEST")
                for i in range(3):
                    nc.vector.tensor_copy(out=osl(V3, i), in_=sl(M9, i))
                nc.vector.tensor_copy(out=NBEST[:], in_=sl(CN, 0))
                for j in (1, 2):
                    tt(Alu.is_gt, GT[:], sl(CN, j), NBEST[:])
                    for i in range(3):
                        tt(Alu.subtract, SC1[:], sl(M9, j * 3 + i), osl(V3, i))
                        tt(Alu.mult, SC1[:], SC1[:], GT[:])
                        tt(Alu.add, osl(V3, i), osl(V3, i), SC1[:])
                    tt(Alu.max, NBEST[:], NBEST[:], sl(CN, j))
                ts(SC1[:], NBEST[:], 1e-37, Alu.max)
                act(SC2[:], SC1[:], Act.Sqrt)
                nc.vector.reciprocal(SC2[:], SC2[:])
                for i in range(3):
                    tt(Alu.mult, osl(V3, i), osl(V3, i), SC2[:])

                tt(Alu.subtract, BEST[:], NVv, sl(OH, 0))
                for k in (1, 2, 3):
                    tt(Alu.subtract, BEST[:], BEST[:], sl(OH, k))
                nc.vector.memset(osl(MODE, 0), 0.0)
                for k in range(1, 5):
                    ck = sl(OH, k - 1)
                    tt(Alu.is_gt, GT[:], ck, BEST[:])
                    nc.vector.tensor_scalar(out=KT[:], in0=osl(MODE, 0),
                                            scalar1=-1.0, scalar2=float(k),
                                            op0=Alu.mult, op1=Alu.add)
                    tt(Alu.mult, KT[:], KT[:], GT[:])
                    tt(Alu.add, osl(MODE, 0), osl(MODE, 0), KT[:])
                    tt(Alu.max, BEST[:], BEST[:], ck)
                return DIRWT

            def pass_b(t):
                ls = F * int(Ob[t])
                xc = xcs[t]
                g = t // GW
                xcx = xc[:, 0:ls]; xcy = xc[:, ls:2 * ls]
                xcz = xc[:, 2 * ls:3 * ls]
                T = sp.tile([P, ls], dt.float32, tag="T", name=f"T{t}")
                S2 = sp.tile([P, ls], dt.float32, tag="S2", name=f"S2_{t}")
                S2b = sp.tile([P, ls], dt.float32, tag="S2b", name=f"S2b{t}")
                R = sp.tile([P, ls], dt.float32, tag="R", name=f"R{t}")
                nc.vector.tensor_scalar(out=T[:], in0=xcx,
                                        scalar1=V3[:, 0 * NT + t:0 * NT + t + 1],
                                        scalar2=None, op0=Alu.mult)
                stt(T[:], xcy, V3[:, 1 * NT + t:1 * NT + t + 1],
                    Alu.mult, Alu.add, T[:])
                stt(T[:], xcz, V3[:, 2 * NT + t:2 * NT + t + 1],
                    Alu.mult, Alu.add, T[:])
                stt(S2[:], xcx, 1.0, Alu.mult, Alu.mult, xcx)
                stt(S2b[:], xcy, 1.0, Alu.mult, Alu.mult, xcy)
                tt(Alu.add, S2[:], S2[:], S2b[:])
                stt(S2b[:], xcz, 1.0, Alu.mult, Alu.mult, xcz)
                tt(Alu.add, S2[:], S2[:], S2b[:])
                stt(S2b[:], T[:], 1.0, Alu.mult, Alu.mult, T[:])
                tt(Alu.subtract, S2[:], S2[:], S2b[:])
                ts(S2[:], S2[:], 0.0, Alu.max)
                act(R[:], S2[:], Act.Sqrt)
                stt(S2b[:], T[:], 1.0, Alu.mult, Alu.mult, R[:],
                    accum=SCRAWg[g][:, t % GW:t % GW + 1])

            def sign_phase(g, DIRWT):
                g0, g1 = g * GW, (g + 1) * GW
                CEN = CENg[g]; SCRAW = SCRAWg[g]
                NVv = NV[:, g0:g1]

                def tmp(tag, k=1):
                    return ret.tile([P, k * GW], dt.float32,
                                    tag=f"{tag}_g{g}", name=f"{tag}_g{g}")

                def sl(T, i):
                    return T[:, i * GW:(i + 1) * GW]

                def osl(T, i):
                    return T[:, i * NT + g0:i * NT + g1]

                T0 = tmp("T0"); CC = tmp("CC"); R0 = tmp("R0")
                SCV = tmp("SCV"); FAC = tmp("FAC"); SC9 = tmp("SC9")
                GT9 = tmp("GT9"); NPAD = tmp("NPAD")
                tt(Alu.mult, T0[:], sl(CEN, 0), osl(V3, 0))
                tt(Alu.mult, SC9[:], sl(CEN, 1), osl(V3, 1))
                tt(Alu.add, T0[:], T0[:], SC9[:])
                tt(Alu.mult, SC9[:], sl(CEN, 2), osl(V3, 2))
                tt(Alu.add, T0[:], T0[:], SC9[:])
                ts(T0[:], T0[:], -1.0, Alu.mult)
                tt(Alu.mult, CC[:], sl(CEN, 0), sl(CEN, 0))
                tt(Alu.mult, SC9[:], sl(CEN, 1), sl(CEN, 1))
                tt(Alu.add, CC[:], CC[:], SC9[:])
                tt(Alu.mult, SC9[:], sl(CEN, 2), sl(CEN, 2))
                tt(Alu.add, CC[:], CC[:], SC9[:])
                tt(Alu.mult, SC9[:], T0[:], T0[:])
                tt(Alu.subtract, R0[:], CC[:], SC9[:])
                ts(R0[:], R0[:], 0.0, Alu.max)
                act(R0[:], R0[:], Act.Sqrt)
                for t in range(g0, g1):
                    nc.vector.tensor_scalar(
                        out=NPAD[:, t - g0:t - g0 + 1],
                        in0=NV[:, t:t + 1], scalar1=-1.0,
                        scalar2=float(F * int(Ob[t])), op0=Alu.mult,
                        op1=Alu.add)
                tt(Alu.mult, SC9[:], T0[:], R0[:])
                tt(Alu.mult, SC9[:], SC9[:], NPAD[:])
                tt(Alu.subtract, SCV[:], SCRAW[:], SC9[:])
                ts(GT9[:], SCV[:], 0.0, Alu.is_lt)
                nc.vector.tensor_scalar(out=GT9[:], in0=GT9[:], scalar1=-2.0,
                                        scalar2=1.0, op0=Alu.mult, op1=Alu.add)
                tt(Alu.mult, FAC[:], DIRWT[:], GT9[:])
                for i in range(3):
                    tt(Alu.mult, osl(V3, i), osl(V3, i), FAC[:])
                for j, pl in [(0, sl(CEN, 0)), (1, sl(CEN, 1)), (2, sl(CEN, 2)),
                              (3, osl(B6, 0)), (4, osl(B6, 1)), (5, osl(B6, 2)),
                              (6, osl(B6, 1)), (7, osl(B6, 3)), (8, osl(B6, 4)),
                              (9, osl(B6, 2)), (10, osl(B6, 4)),
                              (11, osl(B6, 5)),
                              (12, osl(V3, 0)), (13, osl(V3, 1)),
                              (14, osl(V3, 2)),
                              (15, NVv), (16, osl(MEANV, 0)),
                              (17, osl(STDV, 0)),
                              (18, osl(MODE, 0))]:
                    nc.sync.dma_start(out=res[:, j * NT + g0:j * NT + g1],
                                      in_=pl)

            for g in range(NG):
                for t in range(g * GW, (g + 1) * GW):
                    gather_and_pass_a(t)
                DIRWT = cluster_math(g)
                for t in range(g * GW, (g + 1) * GW):
                    pass_b(t)
                sign_phase(g, DIRWT)

    nc.compile()
    return nc


_cache = {}
_last = None


def kernel(data, clust_idx, clust_len):
    global N, C, L, NT
    data = np.asarray(data)
    clust_idx = np.asarray(clust_idx)
    N = int(data.shape[0])
    C, L = int(clust_idx.shape[0]), int(clust_idx.shape[1])
    assert C % (P * N_CORES) == 0, \
        f"cluster count {C} not divisible by {P * N_CORES}"
    NT = C // (P * N_CORES)
    table, idx_blobs, clen_blobs, nvecs, Ob, S, ids = _host_prep(
        data, clust_idx, clust_len)

    key = tuple(int(x) for x in Ob)
    if key not in _cache:
        _cache[key] = _build_program(Ob, S)
    nc = _cache[key]

    from concourse.bass_utils import run_bass_kernel_spmd
    in_maps = [{"table": table, "idx": idx_blobs[c], "clen": clen_blobs[c],
                "nvec": nvecs[c]}
               for c in range(N_CORES)]
    global _last
    _last = (nc, in_maps)
    res = run_bass_kernel_spmd(nc, in_maps, list(range(N_CORES)))

    out = np.zeros((C, 19), dtype=f32)
    for core in range(N_CORES):
        r = res.results[core]["res"].reshape(P, 19, NT)
        for t in range(NT):
            out[ids[core, t]] = r[:, :, t]
    return out
